# revision 1
# baseline (speedup 1.0000x reference)
"""Distributed causal multi-head attention for 8 TRN2 NeuronCores.

Problem: x[2, 2048, 1024], 16 heads x 64 dim, causal softmax attention,
output projection. Sharding: tensor-parallel over (batch, head-group):
core c handles batch c//4 and heads [4*(c%4), 4*(c%4)+4). Each core
computes its 4 heads' attention plus the partial output projection
(sum over its heads); the host sums the 4 partials per batch.

On-device layout strategy (no transposes anywhere on device):
  - host feeds xT = x[b].T               [D=1024, S=2048]
  - wq/wk/wv = W[heads] as [D, 256]      (d-major, head-major columns)
  - wo_h     = W_O slice per head        [64, 1024]
  - Q^T/K^T computed as [head-pair 128, S]; V as [p, 65*4] with a ones
    column folded per head so the attention-value matmul also produces
    the softmax denominator row.
  - scores tile = K^T.T @ Q^T -> [p=128, q=512] in PSUM; causality is
    handled by skipping fully-masked 128-col blocks in scores/exp/AV and
    applying a multiplicative tril [128,128] to the probabilities of the
    true-diagonal blocks after exp (keeps DVE off the ACT feed path).
  - z^T accumulated in PSUM [65, 512] per head (row 64 = denominator l).
  - normalization: r = 1/l broadcast across partitions, z * r -> zn.
  - out[q,1024] = sum_h zn_h.T @ wo_h, accumulated in PSUM over heads.

Matmul compute dtype: bfloat16 (full-rate on TRN2; rel err ~5e-3 vs the
fp32 reference), fp32 accumulation in PSUM. The normalization
outer-product uses float32r to keep the 1/l factors near-fp32 accurate.

Schedule notes (why the structure looks the way it does):
  - Head pairs sit at partition/row-group 0 and 64 everywhere (K^T/Q^T
    layout, O-proj operands): K=64 matmuls that use only half the PE
    array park the HAM activity monitor at the throttled 1.2 GHz clock;
    alternating row groups keeps the array fully active at 2.4 GHz.
  - Scores for a head pair share one 2-bank PSUM tile so a single wide
    ACT exp covers both heads (ACT per-op overhead paced the pipeline).
  - AV matmuls run ~1 p-tile behind scores so the exp latency is off
    the PE critical path.
  - Normalization is deferred: the attention loop ends in plain
    PSUM->SBUF copies (fast z-bank recycling); the slow [128,512]
    reciprocal runs on DVE underneath the next q-chunk's matmuls.
"""

import sys

if "/opt/trn_rl_repo" not in sys.path:
    sys.path.insert(0, "/opt/trn_rl_repo")

import numpy as np

import concourse.bass as bass
import concourse.mybir as mybir
import concourse.tile as tile
from concourse.bass_utils import run_bass_kernel_spmd

B = 2
S = 2048
D = 1024
NH = 16
DH = 64
N_CORES = 8
HPC = 4          # heads per core
HL = HPC * DH    # 256 local head dims
QC = 512         # q-chunk width
NQC = S // QC
NEG = -30000.0   # additive mask value; exp(NEG/8) == 0 in f32

F32 = mybir.dt.float32
F32R = mybir.dt.float32r
BF16 = mybir.dt.bfloat16
EXP = mybir.ActivationFunctionType.Exp


def _split_multiwait(nc, max_waits=1):
    """Walrus (CoreV3) rejects instructions carrying more than one sync
    wait; split extras into single-wait nops inserted before, same engine."""
    for f in nc.m.functions:
        for blk in f.blocks:
            insts = blk.instructions
            idx = 0
            while idx < len(insts):
                inst = insts[idx]
                si = getattr(inst, "sync_info", None)
                waits = list(si.on_wait) if si is not None else []
                if len(waits) > max_waits:
                    extra, keep = waits[:-max_waits], waits[-max_waits:]
                    si.on_wait = keep
                    for j, w in enumerate(extra):
                        nop = mybir.InstNoOp(
                            name=f"{inst.name}_sw{j}",
                            engine=inst.engine,
                            sync_info=mybir.SyncInfo(on_wait=[w], on_update=[]),
                            bass_nofuse=True,
                        )
                        insts.insert(idx, nop)
                        idx += 1
                idx += 1


def build_nc(stage=3):
    """stage 1: projections only (QT dumped to out); 2: + attention loop
    (zn dumped); 3: full kernel."""
    nc = bass.Bass("TRN2", target_bir_lowering=False, debug=False, num_devices=N_CORES)

    xT_d = nc.declare_dram_parameter("xT", [D, S], BF16, isOutput=False)
    wq_d = nc.declare_dram_parameter("wq", [D, HL], BF16, isOutput=False)
    wk_d = nc.declare_dram_parameter("wk", [D, HL], BF16, isOutput=False)
    wv_d = nc.declare_dram_parameter("wv", [D, HL], BF16, isOutput=False)
    wo_d = nc.declare_dram_parameter("wo", [HL, D], BF16, isOutput=False)
    mask_d = nc.declare_dram_parameter("mask", [128, 128], BF16, isOutput=False)
    out_d = nc.declare_dram_parameter("out", [S, D], BF16, isOutput=True)

    with tile.TileContext(nc) as tc:
        with (
            tc.tile_pool(name="live_sb", bufs=1) as live_sb,
            tc.tile_pool(name="att_sb", bufs=1) as att_sb,
        ):
            # Tensors that live through the whole kernel.
            QT = [live_sb.tile([128, S], BF16, tag=f"QT{hc}", name=f"QT{hc}") for hc in range(2)]
            KT = [live_sb.tile([128, S], BF16, tag=f"KT{hc}", name=f"KT{hc}") for hc in range(2)]
            # V with a ones column per head: 16 p-chunks x [V0|1|V1|1|V2|1|V3|1]
            V_sb = live_sb.tile([128, 16 * (HPC * 65)], BF16, tag="V", name="V")
            wop = [live_sb.tile([128, D], BF16, tag=f"wop{hp}", name=f"wop{hp}") for hp in range(2)]
            mask_t = live_sb.tile([128, 128], BF16, tag="mask", name="mask")

            # f32r tiles cannot be memset directly (walrus ISA check); build
            # ones in f32 and round via tensor_copy.
            ones_f = live_sb.tile([128, 64], F32, tag="ones_f", name="ones_f")
            nc.vector.memset(ones_f[:, :], 1.0)
            ones64 = live_sb.tile([1, 64], F32R, tag="ones64", name="ones64")
            nc.vector.tensor_copy(ones64[:, :], ones_f[0:1, :])


            # ---- Phase 1: projections (xT and w tiles scoped here) ----
            with (
                tc.tile_pool(name="xw_sb", bufs=1) as xw_sb,
                tc.tile_pool(name="proj_ps", bufs=4, space="PSUM") as proj_ps,
            ):
                # DMA issue order matters: the projection's first matmul
                # needs wq + xT chunk 0, so weights go first, then x chunks;
                # wv/mask/wo are consumed later.
                w_sb = {}
                w_tiles = {}
                for name in ("wq", "wk", "wv"):
                    w_tiles[name] = xw_sb.tile(
                        [128, 8 * HL], BF16, tag=f"{name}b", name=f"{name}b"
                    )

                def _w_dma(name, dram):
                    t = w_tiles[name]
                    src = dram.ap().rearrange("(di p) h -> di p h", p=128).transpose((1, 0, 2))
                    dst = t[:, :].rearrange("p (di h) -> p di h", di=8)
                    nc.sync.dma_start(out=dst, in_=src)
                    w_sb[name] = t

                _w_dma("wq", wq_d)
                _w_dma("wk", wk_d)
                xT_t = []
                for di in range(8):
                    t = xw_sb.tile([128, S], BF16, tag=f"x{di}", name=f"x{di}")
                    nc.sync.dma_start(out=t[:, :], in_=xT_d[di * 128:(di + 1) * 128, :])
                    xT_t.append(t)
                _w_dma("wv", wv_d)
                nc.sync.dma_start(out=mask_t[:, :], in_=mask_d[:, :])
                for hp in range(2):
                    nc.sync.dma_start(
                        out=wop[hp][:, :], in_=wo_d[hp * 128:(hp + 1) * 128, :]
                    )

                def w_t_slice(name, di, lo, hi):
                    return w_sb[name][:, di * HL + lo:di * HL + hi]

                # Q^T, K^T: [head-pair 128, S]. di outer / qt inner so the
                # stationary weight chunk is reused across 4 matmuls.
                for wname, dst in (("wq", QT), ("wk", KT)):
                    for hc in range(2):
                        pss = [
                            proj_ps.tile([128, 512], F32, tag="pp", name="pp")
                            for _ in range(4)
                        ]
                        for di in range(8):
                            for qt in range(4):
                                nc.tensor.matmul(
                                    pss[qt][:, :],
                                    w_t_slice(wname, di, hc * 128, (hc + 1) * 128),
                                    xT_t[di][:, qt * 512:(qt + 1) * 512],
                                    start=(di == 0),
                                    stop=(di == 7),
                                )
                        for qt in range(4):
                            nc.vector.tensor_copy(
                                dst[hc][:, qt * 512:(qt + 1) * 512], pss[qt][:, :]
                            )

                # V: [p, h] per p-chunk, interleaved with ones columns
                for pc in range(16):
                    ps = proj_ps.tile([128, 512], F32, tag="pp", name="pp")
                    for di in range(8):
                        nc.tensor.matmul(
                            ps[:, :HL],
                            xT_t[di][:, pc * 128:(pc + 1) * 128],
                            w_t_slice("wv", di, 0, HL),
                            start=(di == 0),
                            stop=(di == 7),
                        )
                    base = pc * (HPC * 65)
                    for h in range(HPC):
                        nc.vector.tensor_copy(
                            V_sb[:, base + h * 65: base + h * 65 + 64],
                            ps[:, h * 64:(h + 1) * 64],
                        )
                        nc.gpsimd.tensor_copy(
                            V_sb[:, base + h * 65 + 64: base + h * 65 + 65],
                            ones_f[:, 0:1],
                        )

            if stage == 1:
                dbg = att_sb.tile([128, S], F32, tag="dbg", name="dbg")
                nc.vector.tensor_copy(dbg[:, :], QT[0][:, :])
                nc.sync.dma_start(out=out_d[0:128, :], in_=dbg[:, 0:1024])
                nc.sync.dma_start(out=out_d[128:256, :], in_=dbg[:, 1024:2048])

            # ---- Phase 2: attention (normalization deferred) ----
            # zu: unnormalized z^T per head [64, S]; lall/rall: denominators
            # and their reciprocals, head h parked at partition 32h. The
            # per-qc work ends with plain PSUM->SBUF copies so the z PSUM
            # slots recycle fast and the PE stream never stalls on the
            # (slow, 3.3us) reciprocal, which runs on DVE under the next
            # q-chunk's score matmuls.
            zu = [att_sb.tile([64, S], BF16, tag=f"zu{h}", name=f"zu{h}")
                  for h in range(HPC)]
            lall = att_sb.tile([128, S], F32, tag="lall", name="lall")
            rall = att_sb.tile([128, S], F32, tag="rall", name="rall")
            nc.vector.memset(lall[:, :], 1.0)
            with (
                tc.tile_pool(name="z_ps", bufs=4, space="PSUM") as z_ps,
                tc.tile_pool(name="sc_ps", bufs=2, space="PSUM") as sc_ps,
            ):
                for qc in range(NQC if stage >= 2 else 0):
                    q0 = qc * QC
                    npt = q0 // 128 + 4
                    zt = [z_ps.tile([65, 512], F32, tag="z", name="z") for _ in range(HPC)]
                    # Software pipeline: AV matmuls run one p-tile behind the
                    # score matmuls so ~7 independent PE ops separate a score
                    # from its dependent AV — enough to hide the exp latency.
                    # Scores for a head pair share one 2-bank PSUM tile so a
                    # single wide exp op covers both heads (halves ACT
                    # per-op overhead, which was pacing the pipeline).
                    Ps = {}

                    def emit_scores(pt, hp):
                        p0 = pt * 128
                        jj = pt - q0 // 128  # >=0 means diagonal region
                        # columns [0, jj*128) are fully causal-masked:
                        # skip them in scores, exp and AV entirely.
                        c0 = max(0, jj) * 128
                        scp = sc_ps.tile([128, 1024], F32, tag="sc", name="sc")
                        for i in range(2):
                            h = 2 * hp + i
                            hc, ho = h // 2, (h % 2) * 64
                            nc.tensor.matmul(
                                scp[:, i * 512 + c0:(i + 1) * 512],
                                KT[hc][ho:ho + 64, p0:p0 + 128],
                                QT[hc][ho:ho + 64, q0 + c0:q0 + QC],
                                start=True,
                                stop=True,
                                tile_position=(ho, 0),
                            )
                        Pp = att_sb.tile([128, 1024], BF16, tag="P", name="P", bufs=6)
                        nc.scalar.activation(Pp[:, c0:], scp[:, c0:], EXP, scale=0.125)
                        if jj >= 0:
                            # causal tril applied multiplicatively post-exp:
                            # keeps DVE out of the PE->ACT feed path (ACT
                            # paces the attention pipeline)
                            for i in range(2):
                                blk = slice(i * 512 + jj * 128, i * 512 + (jj + 1) * 128)
                                nc.vector.tensor_mul(Pp[:, blk], Pp[:, blk], mask_t[:, :])
                        Ps[(pt, hp)] = Pp

                    def emit_av(apt, hp):
                        ac0 = max(0, apt - q0 // 128) * 128
                        Pp = Ps.pop((apt, hp))
                        for i in range(2):
                            h = 2 * hp + i
                            nc.tensor.matmul(
                                zt[h][:, ac0:],
                                V_sb[:, apt * (HPC * 65) + h * 65: apt * (HPC * 65) + (h + 1) * 65],
                                Pp[:, i * 512 + ac0:(i + 1) * 512],
                                start=(apt == 0),
                                stop=(apt == npt - 1),
                            )

                    # pair-granular software pipeline, 3-pair (1.5 p-tile) lag
                    steps = [(pt, hp) for pt in range(npt) for hp in range(2)]
                    LAG = 3
                    for n in range(len(steps) + LAG):
                        if n < len(steps):
                            emit_scores(*steps[n])
                        if n >= LAG:
                            emit_av(*steps[n - LAG])

                    last_qc = qc == NQC - 1
                    for h in range(HPC):
                        nc.vector.tensor_copy(
                            lall[32 * h:32 * h + 1, q0:q0 + QC], zt[h][64:65, :]
                        )
                        # final q-chunk: ACT is idle after the last exps, so
                        # splitting the z-drain there releases the PSUM banks
                        # (and thus phase 3's first matmuls) sooner
                        eng = nc.scalar if (last_qc and h >= 2) else nc.vector
                        if eng is nc.scalar:
                            eng.copy(zu[h][:, q0:q0 + QC], zt[h][0:64, :])
                        else:
                            eng.tensor_copy(zu[h][:, q0:q0 + QC], zt[h][0:64, :])
                    nc.vector.reciprocal(
                        rall[:, q0:q0 + QC], lall[:, q0:q0 + QC]
                    )

            # ---- Phase 3: normalization + output projection ----
            # (z/sc pools closed above so these banks are free again)
            # All 16 normalization chains first (the K=1 broadcast
            # matmuls would otherwise chill the PE between O-proj
            # groups), then the O-projection runs as one dense block.
            with tc.tile_pool(name="rb_ps_pool", bufs=4, space="PSUM") as rb_pool:
                rbs = {}
                for qc in range(NQC if stage >= 3 else 0):
                    q0 = qc * QC
                    for h in range(HPC):
                        r_sb = att_sb.tile([1, 512], F32R, tag="r", name="r", bufs=4)
                        nc.gpsimd.tensor_copy(
                            r_sb[:, :], rall[32 * h:32 * h + 1, q0:q0 + QC]
                        )
                        rb_ps = rb_pool.tile([64, 512], F32, tag="rbp", name="rb_ps")
                        nc.tensor.matmul(
                            rb_ps[:, :], ones64[:, :], r_sb[:, :],
                            start=True, stop=True,
                        )
                        rb = att_sb.tile([64, 512], F32, tag="rb", name=f"rb", bufs=16)
                        if h % 2 == 0:
                            nc.scalar.copy(rb[:, :], rb_ps[:, :])
                        else:
                            nc.vector.tensor_copy(rb[:, :], rb_ps[:, :])
                        rbs[(qc, h)] = rb

            with tc.tile_pool(name="o_ps", bufs=8, space="PSUM") as o_ps:
                znps = {}
                for qc in range(NQC if stage >= 3 else 0):
                    q0 = qc * QC
                    znp = [att_sb.tile([128, 512], BF16, tag=f"znp{hp}",
                                       name=f"znp{hp}", bufs=6)
                           for hp in range(2)]
                    for h in range(HPC):
                        hp, off = h // 2, (h % 2) * 64
                        nc.gpsimd.tensor_mul(
                            znp[hp][off:off + 64, :],
                            zu[h][:, q0:q0 + QC], rbs[(qc, h)][:, :],
                        )
                    znps[qc] = znp

                for qc in range(NQC if stage >= 3 else 0):
                    q0 = qc * QC
                    znp = znps[qc]

                    # out[q0:q0+512, :] = sum_h zn_h.T @ wo_h; head pairs sit
                    # at row groups 0/64 so the K=64 matmuls keep the full
                    # array active (HAM stays warm).
                    for qs in range(4):
                        ot = att_sb.tile([128, 1024], BF16, tag="ot", name="ot", bufs=4)
                        for dm in range(2):
                            # one PSUM bank per row group: concurrent
                            # row-packed matmuls must not share a bank
                            ps2 = [o_ps.tile([128, 512], F32, tag="o", name="o")
                                   for _ in range(2)]
                            for hp in range(2):
                                for i in range(2):
                                    nc.tensor.matmul(
                                        ps2[i][:, :],
                                        znp[hp][64 * i:64 * i + 64, qs * 128:(qs + 1) * 128],
                                        wop[hp][64 * i:64 * i + 64, dm * 512:(dm + 1) * 512],
                                        start=(hp == 0),
                                        stop=(hp == 1),
                                        tile_position=(64 * i, 0),
                                    )
                            mg = att_sb.tile([128, 512], F32, tag="mg", name="mg", bufs=6)
                            nc.scalar.copy(mg[:, :], ps2[0][:, :])
                            nc.vector.tensor_add(
                                ot[:, dm * 512:(dm + 1) * 512], ps2[1][:, :], mg[:, :]
                            )
                        nc.sync.dma_start(
                            out=out_d[q0 + qs * 128: q0 + (qs + 1) * 128, :],
                            in_=ot[:, :],
                        )

    _split_multiwait(nc)
    return nc


def _prep_in_maps(x, W_K, W_Q, W_V, W_O):
    x = np.asarray(x, dtype=np.float32)
    W_K = np.asarray(W_K, dtype=np.float32)
    W_Q = np.asarray(W_Q, dtype=np.float32)
    W_V = np.asarray(W_V, dtype=np.float32)
    W_O = np.asarray(W_O, dtype=np.float32)

    import ml_dtypes
    bf16 = ml_dtypes.bfloat16
    pp, qq = np.meshgrid(np.arange(128), np.arange(128), indexing="ij")
    mask = np.where(qq >= pp, 1.0, 0.0).astype(bf16)

    in_maps = []
    for c in range(N_CORES):
        b, g = c // 4, c % 4
        hs = slice(HPC * g, HPC * g + HPC)
        xT = np.ascontiguousarray(x[b].T).astype(bf16)
        wq = np.ascontiguousarray(W_Q[hs].transpose(2, 0, 1).reshape(D, HL)).astype(bf16)
        wk = np.ascontiguousarray(W_K[hs].transpose(2, 0, 1).reshape(D, HL)).astype(bf16)
        wv = np.ascontiguousarray(W_V[hs].transpose(2, 0, 1).reshape(D, HL)).astype(bf16)
        wo = np.ascontiguousarray(W_O[:, HL * g:HL * g + HL].T).astype(bf16)
        in_maps.append(
            {"xT": xT, "wq": wq, "wk": wk, "wv": wv, "wo": wo, "mask": mask}
        )
    return in_maps


_NC_CACHE = None


def _get_nc():
    global _NC_CACHE
    if _NC_CACHE is None:
        _NC_CACHE = build_nc()
    return _NC_CACHE


def _run(x, W_K, W_Q, W_V, W_O, trace=False):
    nc = _get_nc()
    in_maps = _prep_in_maps(x, W_K, W_Q, W_V, W_O)
    res = run_bass_kernel_spmd(
        nc, in_maps, core_ids=list(range(N_CORES)), trace=trace
    )
    partials = np.stack(
        [np.asarray(res.results[c]["out"]).astype(np.float32) for c in range(N_CORES)]
    )
    out = np.empty((B, S, D), dtype=np.float32)
    out[0] = partials[0:4].sum(axis=0)
    out[1] = partials[4:8].sum(axis=0)
    return out, res


def kernel(x, W_K, W_Q, W_V, W_O):
    out, _ = _run(x, W_K, W_Q, W_V, W_O, trace=False)
    return out


def run_traced(x, W_K, W_Q, W_V, W_O):
    """For test.py: returns (out, BassKernelResults with exec_time_ns)."""
    import types

    if "antenv.axon_hooks" not in sys.modules:
        try:
            from trn_agent_boot.trn_boot import _ntff_profile_via_ctypes

            hook = _ntff_profile_via_ctypes("/opt/axon/libaxon_pjrt.so")
            mod = types.ModuleType("antenv.axon_hooks")
            mod.get_axon_ntff_profile_hook = lambda: hook
            mod.set_axon_ntff_profile_hook = lambda h: None
            sys.modules["antenv.axon_hooks"] = mod
        except Exception:
            pass
    return _run(x, W_K, W_Q, W_V, W_O, trace=True)



# revision 12
# speedup vs baseline: 1.2325x; 1.2325x over previous
"""Distributed causal multi-head attention for 8 TRN2 NeuronCores.

Problem: x[2, 2048, 1024], 16 heads x 64 dim, causal softmax attention,
output projection. Sharding: tensor-parallel over (batch, head-group):
core c handles batch c//4 and heads [4*(c%4), 4*(c%4)+4). Each core
computes its 4 heads' attention plus the partial output projection
(sum over its heads); the host sums the 4 partials per batch.

On-device layout strategy (no transposes anywhere on device):
  - host feeds xT = x[b].T               [D=1024, S=2048]
  - wq/wk/wv = W[heads] as [D, 256]      (d-major, head-major columns)
  - wo pair  = W_O rows per head pair    [128, 1024]
  - Q^T/K^T computed as [head-pair 128, S]; V as [p, 65*4] with a ones
    column folded per head so the attention-value matmul also produces
    the softmax denominator row.
  - scores tile per (p-tile, head pair) is one 2-bank PSUM tile laid
    [h0 | h1] with the causally-dead c0 columns squeezed out so a single
    contiguous ACT exp covers both heads with no garbage columns (ACT
    paces the attention pipeline; every element counts).
  - causal handling: fully-masked 128-col blocks are skipped in
    scores/exp/AV; the true-diagonal 128x128 block gets a multiplicative
    tril on the probabilities after exp (keeps DVE off the ACT path).
  - z^T accumulated in PSUM [65, 512] per head (row 64 = denominator l).
  - normalization (deferred one q-chunk, hidden under the next chunk's
    attention): l rows gathered to [4, S], reciprocal + bf16 cast on DVE,
    then one K=2 indicator matmul per head pair broadcasts r across 128
    partitions into a recycled z PSUM bank; DVE muls produce the
    normalized pair tile zup[hp] [128, S] consumed by the O-projection.
  - O-projection: out[q,1024] accumulates TWO K=128 matmuls per output
    tile (head pairs stacked on the contraction axis) - full PE array,
    half the instruction count of per-head K=64 matmuls.

Matmul compute dtype: bfloat16 (full-rate on TRN2; rel err ~6e-3 vs the
fp32 reference), fp32 accumulation in PSUM. The per-q 1/l factors are
bf16 (adds <0.4% rms; the 2e-2 gate has plenty of margin).

Schedule notes:
  - Score matmuls for the two heads of a pair sit at PE row groups 0/64
    (tile_position) and execute CONCURRENTLY on the array - confirmed
    from the perfetto trace (pairs overlap ~90%).
  - AV matmuls run LAG=3 pair-steps behind scores to hide exp latency.
  - The deferred normalization emits its two rb matmuls right after the
    next q-chunk's first score pair so the PE never waits on the DVE
    reciprocal chain; the z PSUM ring (bufs=4) naturally recycles
    drained banks for them.
"""

import sys

if "/opt/trn_rl_repo" not in sys.path:
    sys.path.insert(0, "/opt/trn_rl_repo")

import numpy as np

import concourse.bass as bass
import concourse.mybir as mybir
import concourse.tile as tile
from concourse.bass_utils import run_bass_kernel_spmd

B = 2
S = 2048
D = 1024
NH = 16
DH = 64
N_CORES = 8
HPC = 4          # heads per core
HL = HPC * DH    # 256 local head dims
QC = 512         # q-chunk width
NQC = S // QC

F32 = mybir.dt.float32
BF16 = mybir.dt.bfloat16
EXP = mybir.ActivationFunctionType.Exp


def _split_multiwait(nc, max_waits=1):
    """Walrus (CoreV3) rejects instructions carrying more than one sync
    wait; split extras into single-wait nops inserted before, same engine."""
    for f in nc.m.functions:
        for blk in f.blocks:
            insts = blk.instructions
            idx = 0
            while idx < len(insts):
                inst = insts[idx]
                si = getattr(inst, "sync_info", None)
                waits = list(si.on_wait) if si is not None else []
                if len(waits) > max_waits:
                    extra, keep = waits[:-max_waits], waits[-max_waits:]
                    si.on_wait = keep
                    for j, w in enumerate(extra):
                        nop = mybir.InstNoOp(
                            name=f"{inst.name}_sw{j}",
                            engine=inst.engine,
                            sync_info=mybir.SyncInfo(on_wait=[w], on_update=[]),
                            bass_nofuse=True,
                        )
                        insts.insert(idx, nop)
                        idx += 1
                idx += 1


def build_nc():
    nc = bass.Bass("TRN2", target_bir_lowering=False, debug=False, num_devices=N_CORES)

    xT_d = nc.declare_dram_parameter("xT", [D, S], BF16, isOutput=False)
    wq_d = nc.declare_dram_parameter("wq", [D, HL], BF16, isOutput=False)
    wk_d = nc.declare_dram_parameter("wk", [D, HL], BF16, isOutput=False)
    wv_d = nc.declare_dram_parameter("wv", [D, HL], BF16, isOutput=False)
    wo_d = nc.declare_dram_parameter("wo", [HL, D], BF16, isOutput=False)
    mask_d = nc.declare_dram_parameter("mask", [128, 128], BF16, isOutput=False)
    out_d = nc.declare_dram_parameter("out", [S, D], BF16, isOutput=True)

    with tile.TileContext(nc) as tc:
        with (
            tc.tile_pool(name="live_sb", bufs=1) as live_sb,
            tc.tile_pool(name="att_sb", bufs=1) as att_sb,
        ):
            # Tensors that live through the whole kernel.
            QT = [live_sb.tile([128, S], BF16, tag=f"QT{hc}", name=f"QT{hc}") for hc in range(2)]
            KT = [live_sb.tile([128, S], BF16, tag=f"KT{hc}", name=f"KT{hc}") for hc in range(2)]
            # V with a ones column per head: 16 p-chunks x [V0|1|V1|1|V2|1|V3|1]
            V_sb = live_sb.tile([128, 16 * (HPC * 65)], BF16, tag="V", name="V")
            wop = [live_sb.tile([128, D], BF16, tag=f"wop{hp}", name=f"wop{hp}") for hp in range(2)]
            mask_t = live_sb.tile([128, 128], BF16, tag="mask", name="mask")

            # Indicator weights for the r broadcast matmul. Head h's 1/l
            # row lives at partition 32h (engine ops need 32-aligned
            # bases). ind_hp is a one-hot [128, 128] selecting partition
            # 64hp -> output rows 0-63 and 64hp+32 -> rows 64-127.
            indf = live_sb.tile([128, 128], F32, tag="indf", name="indf")
            ind = [live_sb.tile([128, 128], BF16, tag=f"ind{hp}", name=f"ind{hp}")
                   for hp in range(2)]
            for hp in range(2):
                nc.vector.memset(indf[:, :], 0.0)
                nc.vector.memset(indf[64 * hp:64 * hp + 1, 0:64], 1.0)
                nc.vector.memset(indf[64 * hp + 32:64 * hp + 33, 64:128], 1.0)
                nc.vector.tensor_copy(ind[hp][:, :], indf[:, :])

            # ones columns of V (col 64 of each head block), one strided memset
            ones_ap = V_sb[:, :].rearrange("p (a c) -> p a c", c=65)[:, :, 64:65]
            nc.vector.memset(ones_ap, 1.0)

            # ---- Phase 1: projections (xT and w tiles scoped here) ----
            with (
                tc.tile_pool(name="xw_sb", bufs=1) as xw_sb,
                tc.tile_pool(name="proj_ps", bufs=4, space="PSUM") as proj_ps,
            ):
                # DMA issue order matters: the projection's first matmul
                # needs wq + xT chunk 0, so those go first; wk before the
                # x tail so the K^T loop never waits on it.
                w_sb = {}
                w_tiles = {}
                for name in ("wq", "wk", "wv"):
                    w_tiles[name] = xw_sb.tile(
                        [128, 8 * HL], BF16, tag=f"{name}b", name=f"{name}b"
                    )

                def _w_dma(name, dram):
                    t = w_tiles[name]
                    src = dram.ap().rearrange("(di p) h -> di p h", p=128).transpose((1, 0, 2))
                    dst = t[:, :].rearrange("p (di h) -> p di h", di=8)
                    nc.sync.dma_start(out=dst, in_=src)
                    w_sb[name] = t

                xT_t = [
                    xw_sb.tile([128, S], BF16, tag=f"x{di}", name=f"x{di}")
                    for di in range(8)
                ]

                def _x_dma(di):
                    nc.sync.dma_start(
                        out=xT_t[di][:, :], in_=xT_d[di * 128:(di + 1) * 128, :]
                    )

                _w_dma("wq", wq_d)
                _x_dma(0)
                _x_dma(1)
                _w_dma("wk", wk_d)
                for di in range(2, 8):
                    _x_dma(di)
                _w_dma("wv", wv_d)
                nc.sync.dma_start(out=mask_t[:, :], in_=mask_d[:, :])
                for hp in range(2):
                    nc.sync.dma_start(
                        out=wop[hp][:, :], in_=wo_d[hp * 128:(hp + 1) * 128, :]
                    )

                def w_t_slice(name, di, lo, hi):
                    return w_sb[name][:, di * HL + lo:di * HL + hi]

                # Q^T, K^T: [head-pair 128, S]. di outer / qt inner so the
                # stationary weight chunk is reused across 4 matmuls.
                # PSUM drains split vector/scalar (ACT is idle in phase 1).
                ndr = 0
                for wname, dst in (("wq", QT), ("wk", KT)):
                    for hc in range(2):
                        pss = [
                            proj_ps.tile([128, 512], F32, tag="pp", name="pp")
                            for _ in range(4)
                        ]
                        for di in range(8):
                            for qt in range(4):
                                nc.tensor.matmul(
                                    pss[qt][:, :],
                                    w_t_slice(wname, di, hc * 128, (hc + 1) * 128),
                                    xT_t[di][:, qt * 512:(qt + 1) * 512],
                                    start=(di == 0),
                                    stop=(di == 7),
                                )
                        for qt in range(4):
                            eng = nc.vector if ndr % 2 == 0 else nc.scalar
                            if eng is nc.scalar:
                                eng.copy(
                                    dst[hc][:, qt * 512:(qt + 1) * 512], pss[qt][:, :]
                                )
                            else:
                                eng.tensor_copy(
                                    dst[hc][:, qt * 512:(qt + 1) * 512], pss[qt][:, :]
                                )
                            ndr += 1

                # V: [p, h] per p-chunk; one strided drain per chunk writes
                # around the pre-set ones columns.
                for pc in range(16):
                    ps = proj_ps.tile([128, 512], F32, tag="pp", name="pp")
                    for di in range(8):
                        nc.tensor.matmul(
                            ps[:, :HL],
                            xT_t[di][:, pc * 128:(pc + 1) * 128],
                            w_t_slice("wv", di, 0, HL),
                            start=(di == 0),
                            stop=(di == 7),
                        )
                    base = pc * (HPC * 65)
                    vdst = V_sb[:, base:base + HPC * 65].rearrange(
                        "p (h c) -> p h c", c=65
                    )[:, :, 0:64]
                    vsrc = ps[:, :HL].rearrange("p (h c) -> p h c", c=64)
                    if pc % 2 == 0:
                        nc.vector.tensor_copy(vdst, vsrc)
                    else:
                        nc.scalar.copy(vdst, vsrc)

            # ---- Phase 2: attention (normalization deferred 1 q-chunk) ----
            # zu: unnormalized z^T per head [64, S]; lall row h = denominator
            # of head h; rqb = bf16(1/l). zup: normalized head-pair tiles
            # [128, S] feeding the K=128 O-projection.
            zu = [att_sb.tile([64, S], BF16, tag=f"zu{h}", name=f"zu{h}")
                  for h in range(HPC)]
            zup = [att_sb.tile([128, S], BF16, tag=f"zup{hp}", name=f"zup{hp}")
                   for hp in range(2)]
            # head h's denominator / reciprocal parked at partition 32h
            lall = att_sb.tile([128, S], F32, tag="lall", name="lall")
            rall = att_sb.tile([128, S], F32, tag="rall", name="rall")
            rqb = att_sb.tile([128, S], BF16, tag="rqb", name="rqb")
            nc.vector.memset(lall[:, :], 1.0)
            with (
                tc.tile_pool(name="z_ps", bufs=4, space="PSUM") as z_ps,
                tc.tile_pool(name="sc_ps", bufs=2, space="PSUM") as sc_ps,
            ):
                pending_norm = None

                for qc in range(NQC):
                    q0 = qc * QC
                    npt = q0 // 128 + 4
                    zt = []  # allocated lazily at the first AV step
                    Ps = {}

                    def emit_scores(pt, hp):
                        p0 = pt * 128
                        jj = pt - q0 // 128  # >=0 means diagonal region
                        c0 = max(0, jj) * 128
                        # pair layout [h0 | h1] with dead columns squeezed:
                        # h0 -> cols [c0:512], h1 -> cols [512:1024-c0], so
                        # one contiguous exp covers both heads, no garbage.
                        scp = sc_ps.tile([128, 1024], F32, tag="sc", name="sc")
                        for i in range(2):
                            h = 2 * hp + i
                            hc, ho = h // 2, (h % 2) * 64
                            dst = (scp[:, c0:512] if i == 0
                                   else scp[:, 512:1024 - c0])
                            nc.tensor.matmul(
                                dst,
                                KT[hc][ho:ho + 64, p0:p0 + 128],
                                QT[hc][ho:ho + 64, q0 + c0:q0 + QC],
                                start=True,
                                stop=True,
                                tile_position=(ho, 0),
                            )
                        Pp = att_sb.tile([128, 1024], BF16, tag="P", name="P", bufs=6)
                        nc.scalar.activation(
                            Pp[:, c0:1024 - c0], scp[:, c0:1024 - c0], EXP,
                            scale=0.125,
                        )
                        if jj >= 0:
                            # causal tril applied multiplicatively post-exp
                            blk0 = slice(jj * 128, (jj + 1) * 128)
                            nc.vector.tensor_mul(Pp[:, blk0], Pp[:, blk0], mask_t[:, :])
                            nc.vector.tensor_mul(Pp[:, 512:640], Pp[:, 512:640], mask_t[:, :])
                        Ps[(pt, hp)] = Pp

                    def emit_av(apt, hp):
                        ac0 = max(0, apt - q0 // 128) * 128
                        Pp = Ps.pop((apt, hp))
                        if not zt:
                            zt.extend(
                                z_ps.tile([128, 512], F32, tag="z", name="z")
                                for _ in range(HPC)
                            )
                        for i in range(2):
                            h = 2 * hp + i
                            src = (Pp[:, ac0:512] if i == 0
                                   else Pp[:, 512:1024 - ac0])
                            nc.tensor.matmul(
                                zt[h][0:65, ac0:],
                                V_sb[:, apt * (HPC * 65) + h * 65: apt * (HPC * 65) + (h + 1) * 65],
                                src,
                                start=(apt == 0),
                                stop=(apt == npt - 1),
                            )

                    # pair-granular software pipeline, 3-pair lag so the exp
                    # latency stays off the PE critical path. The previous
                    # chunk's deferred normalization slots in after the first
                    # score pair (its rb matmuls then never stall the PE).
                    steps = [(pt, hp) for pt in range(npt) for hp in range(2)]
                    LAG = 3
                    for n in range(len(steps) + LAG):
                        if n < len(steps):
                            emit_scores(*steps[n])
                        if n == 1 and pending_norm is not None:
                            pending_norm()
                            pending_norm = None
                        if n >= LAG:
                            emit_av(*steps[n - LAG])

                    # ---- q-chunk tail: gather l, drain z, 1/l in bf16 ----
                    # (gpsimd cannot read PSUM: all PSUM drains on vector,
                    # SBUF-to-SBUF cast on gpsimd)
                    for h in range(HPC):
                        nc.vector.tensor_copy(
                            lall[32 * h:32 * h + 1, q0:q0 + QC], zt[h][64:65, 0:512]
                        )
                    nc.vector.reciprocal(rall[:, q0:q0 + QC], lall[:, q0:q0 + QC])
                    nc.gpsimd.tensor_copy(rqb[:, q0:q0 + QC], rall[:, q0:q0 + QC])
                    for h in range(HPC):
                        nc.vector.tensor_copy(zu[h][:, q0:q0 + QC], zt[h][0:64, 0:512])

                    def make_norm(q0):
                        def _norm():
                            for hp in range(2):
                                rbt = z_ps.tile([128, 512], F32, tag="z", name="z")
                                nc.tensor.matmul(
                                    rbt[:, :], ind[hp][:, :],
                                    rqb[:, q0:q0 + QC],
                                    start=True, stop=True,
                                )
                                for i in range(2):
                                    nc.vector.tensor_mul(
                                        zup[hp][64 * i:64 * i + 64, q0:q0 + QC],
                                        zu[2 * hp + i][:, q0:q0 + QC],
                                        rbt[64 * i:64 * i + 64, :],
                                    )
                        return _norm

                    pending_norm = make_norm(q0)

                # last q-chunk's normalization (z pool still open: rb
                # matmuls recycle the drained z banks)
                pending_norm()
                pending_norm = None

            # ---- Phase 3: output projection, K=128 head-pair matmuls ----
            with tc.tile_pool(name="o_ps", bufs=4, space="PSUM") as o_ps:
                not_ = 0
                for qc in range(NQC):
                    q0 = qc * QC
                    for qs in range(4):
                        pso = o_ps.tile([128, 1024], F32, tag="o", name="o")
                        for dm in range(2):
                            for hp in range(2):
                                nc.tensor.matmul(
                                    pso[:, dm * 512:(dm + 1) * 512],
                                    zup[hp][:, q0 + qs * 128:q0 + (qs + 1) * 128],
                                    wop[hp][:, dm * 512:(dm + 1) * 512],
                                    start=(hp == 0),
                                    stop=(hp == 1),
                                )
                        ot = att_sb.tile([128, D], BF16, tag="ot", name="ot", bufs=4)
                        if not_ % 2 == 0:
                            nc.scalar.copy(ot[:, :], pso[:, :])
                        else:
                            nc.vector.tensor_copy(ot[:, :], pso[:, :])
                        not_ += 1
                        nc.sync.dma_start(
                            out=out_d[q0 + qs * 128: q0 + (qs + 1) * 128, :],
                            in_=ot[:, :],
                        )

    _split_multiwait(nc)
    return nc


def _prep_in_maps(x, W_K, W_Q, W_V, W_O):
    x = np.asarray(x, dtype=np.float32)
    W_K = np.asarray(W_K, dtype=np.float32)
    W_Q = np.asarray(W_Q, dtype=np.float32)
    W_V = np.asarray(W_V, dtype=np.float32)
    W_O = np.asarray(W_O, dtype=np.float32)

    import ml_dtypes
    bf16 = ml_dtypes.bfloat16
    pp, qq = np.meshgrid(np.arange(128), np.arange(128), indexing="ij")
    mask = np.where(qq >= pp, 1.0, 0.0).astype(bf16)

    in_maps = []
    for c in range(N_CORES):
        b, g = c // 4, c % 4
        hs = slice(HPC * g, HPC * g + HPC)
        xT = np.ascontiguousarray(x[b].T).astype(bf16)
        wq = np.ascontiguousarray(W_Q[hs].transpose(2, 0, 1).reshape(D, HL)).astype(bf16)
        wk = np.ascontiguousarray(W_K[hs].transpose(2, 0, 1).reshape(D, HL)).astype(bf16)
        wv = np.ascontiguousarray(W_V[hs].transpose(2, 0, 1).reshape(D, HL)).astype(bf16)
        wo = np.ascontiguousarray(W_O[:, HL * g:HL * g + HL].T).astype(bf16)
        in_maps.append(
            {"xT": xT, "wq": wq, "wk": wk, "wv": wv, "wo": wo, "mask": mask}
        )
    return in_maps


_NC_CACHE = None


def _get_nc():
    global _NC_CACHE
    if _NC_CACHE is None:
        _NC_CACHE = build_nc()
    return _NC_CACHE


def _run(x, W_K, W_Q, W_V, W_O, trace=False):
    nc = _get_nc()
    in_maps = _prep_in_maps(x, W_K, W_Q, W_V, W_O)
    res = run_bass_kernel_spmd(
        nc, in_maps, core_ids=list(range(N_CORES)), trace=trace
    )
    partials = np.stack(
        [np.asarray(res.results[c]["out"]).astype(np.float32) for c in range(N_CORES)]
    )
    out = np.empty((B, S, D), dtype=np.float32)
    out[0] = partials[0:4].sum(axis=0)
    out[1] = partials[4:8].sum(axis=0)
    return out, res


def kernel(x, W_K, W_Q, W_V, W_O):
    out, _ = _run(x, W_K, W_Q, W_V, W_O, trace=False)
    return out


def run_traced(x, W_K, W_Q, W_V, W_O):
    """For test.py: returns (out, BassKernelResults with exec_time_ns)."""
    import types

    if "antenv.axon_hooks" not in sys.modules:
        try:
            from trn_agent_boot.trn_boot import _ntff_profile_via_ctypes

            hook = _ntff_profile_via_ctypes("/opt/axon/libaxon_pjrt.so")
            mod = types.ModuleType("antenv.axon_hooks")
            mod.get_axon_ntff_profile_hook = lambda: hook
            mod.set_axon_ntff_profile_hook = lambda h: None
            sys.modules["antenv.axon_hooks"] = mod
        except Exception:
            pass
    return _run(x, W_K, W_Q, W_V, W_O, trace=True)
